# revision 1
# baseline (speedup 1.0000x reference)
"""StyleGAN2-style modulated 3x3 conv (B=16, C=128, H=W=128) on 8 TRN2 NeuronCores.

Sharding: data-parallel over batch (2 samples/core). The grouped conv runs as 9
accumulated matmuls per 4-row PSUM tile with the input-channel dim (128) as the
contraction. Spatial zero-padding is never materialized: boundary taps issue
partial-region PSUM accumulations instead, so band DMAs are fully contiguous.

Weight normalization (1/(sqrt(i*k*k) * ||w||_inf)) is folded into the per-
(sample, out-channel) demodulation scale applied at PSUM drain, so the matmul
weights are just raw-transposed weights modulated by the per-input-channel
style. Drains run on the scalar/gpsimd engines, keeping vector off the
critical path.
"""

import math
from itertools import product

import numpy as np

import concourse.bacc as bacc
import concourse.bass as bass
import concourse.mybir as mybir
import concourse.tile as tile
from concourse.bass_utils import run_bass_kernel_spmd
from concourse.masks import make_identity

B, C, H, W = 16, 128, 128, 128
KK = 3
EPS = 1e-8
N_CORES = 8
S = B // N_CORES          # samples per core
RPT = 4                   # output rows per PSUM tile (one PSUM bank)
BH = 32                   # output rows per band
NB = H // BH              # bands per sample
BROWS = BH + 2            # band rows incl. 1-row halo each side
WP = W + 2                # band width incl. 1-col zero pad each side
GT = 4                    # PSUM tiles per output store DMA
NKK = KK * KK

FP32 = mybir.dt.float32
FP32R = mybir.dt.float32r
BF16 = mybir.dt.bfloat16

BF16_MM = False           # measured slower: bf16 mm = 259ns/512 rows vs fp32r 236ns


def build_bass() -> bass.Bass:
    w_dt = BF16 if BF16_MM else FP32R
    nc = bacc.Bacc(None)
    x_d = nc.dram_tensor("x", [S, C, H, W], FP32R, kind="ExternalInput")
    style_d = nc.dram_tensor("style", [S, C], FP32, kind="ExternalInput")
    w_d = nc.dram_tensor("weight", [C, C, KK, KK], FP32, kind="ExternalInput")
    out_d = nc.dram_tensor("out", [S, C, H, W], FP32, kind="ExternalOutput")

    with tile.TileContext(nc) as tc:
        with (
            tc.tile_pool(name="const", bufs=1) as const_pool,
            tc.tile_pool(name="wpool", bufs=1) as wpool,
            tc.tile_pool(name="xpool", bufs=4) as xpool,
            tc.tile_pool(name="xbf", bufs=4) as xbf,
            tc.tile_pool(name="opool", bufs=3) as opool,
            tc.tile_pool(name="dram", bufs=1, space="DRAM") as dram_pool,
            tc.tile_pool(name="psum_conv", bufs=4, space="PSUM") as psum_conv,
            tc.tile_pool(name="psum_t", bufs=2, space="PSUM") as psum_t,
            tc.tile_pool(name="psum_misc", bufs=1, space="PSUM") as psum_misc,
        ):
            zeros = const_pool.tile([128, BROWS], FP32)
            nc.gpsimd.memset(zeros[:], 0.0)

            # ---- input DMAs (kick off immediately) ----
            # The weight load goes through the gpsimd software DGE so it
            # transfers in parallel with the SP queue, whose order is the
            # latency order: style row, leading rows of band 0 (they gate the
            # first conv matmuls), rest of band 0.
            srow = wpool.tile([S, 2 * C], FP32)
            nc.sync.dma_start(srow[:, 0:C], style_d[:])
            Wt = wpool.tile([C, C * NKK], FP32)
            nc.gpsimd.dma_start(Wt[:], w_d[:].rearrange("o i kh kw -> o (i kh kw)"))
            band0 = xpool.tile([C, BROWS, WP], FP32R, name="band", tag="band")
            nc.gpsimd.tensor_copy(band0[:, :, 0], zeros[:])
            nc.gpsimd.tensor_copy(band0[:, :, WP - 1], zeros[:])
            nc.sync.dma_start(band0[:, 1:10, 1:W + 1], x_d[0, :, 0:9, :])
            for ja, jb in ((10, 22), (22, 34)):
                nc.sync.dma_start(band0[:, ja:jb, 1:W + 1], x_d[0, :, ja - 1:jb - 1, :])
            if BF16_MM:
                band0bf = xbf.tile([C, BROWS, WP], BF16, name="bandbf", tag="bandbf")
                nc.gpsimd.tensor_copy(band0bf[:, 0:1, :], band0[:, 0:1, :])
                for ja, jb in ((1, 10), (10, 22), (22, 34)):
                    nc.gpsimd.tensor_copy(band0bf[:, ja:jb, :], band0[:, ja:jb, :])

            ident = const_pool.tile([128, 128], FP32)
            make_identity(nc, ident)

            # taps ordered so (1,1) — always full-size — comes first: it is the
            # start=True matmul of every accumulation group, and its weight
            # block is transposed/modulated first so the conv can start early.
            TAPS = [(1, 1)] + [t for t in product(range(KK), range(KK)) if t != (1, 1)]

            # ---- style path (vector; independent of weights) ----
            smax = wpool.tile([S, 1], FP32)
            nc.vector.tensor_reduce(
                smax[:], srow[:, 0:C], axis=mybir.AxisListType.X,
                op=mybir.AluOpType.max, apply_absolute_value=True,
            )
            sinv = wpool.tile([S, 1], FP32)
            nc.vector.reciprocal(sinv[:], smax[:])
            nc.vector.tensor_scalar_mul(srow[:, 0:C], srow[:, 0:C], sinv[:])
            nc.vector.tensor_mul(srow[:, C:2 * C], srow[:, 0:C], srow[:, 0:C])
            # on-chip transpose to column layout (the DMA queues are saturated
            # with band loads at this point, so no DRAM bounce):
            # scol[c, 0:S] = s[b, c], scol[c, S:2S] = s[b, c]^2
            scol = wpool.tile([C, 2 * S], FP32)
            for h in range(2):
                pt_s = psum_misc.tile([C, S], FP32, name=f"pts{h}", tag="pts")
                nc.tensor.transpose(
                    pt_s[:], srow[:, h * C:(h + 1) * C], ident[0:S, 0:S],
                )
                nc.scalar.activation(
                    scol[:, h * S:(h + 1) * S], pt_s[:],
                    mybir.ActivationFunctionType.Copy,
                )

            # ---- transpose raw weights tap by tap: W_t[i, k*C+o] ----
            # Each tap's transposed block is immediately modulated by the
            # sample-0 style on vector, so the conv matmuls (which consume the
            # taps in the same order) start as soon as tap (1,1) is ready.
            W_t = wpool.tile([C, NKK * C], FP32)
            wmod = [
                wpool.tile([C, NKK * C], w_dt, name=f"wmod{b}", tag=f"wmod{b}")
                for b in range(S)
            ]
            Wt_koi = Wt[:].rearrange("o (i k) -> o k i", k=NKK)
            wsq = wpool.tile([C, NKK * C], FP32)
            for dy, dx in TAPS:
                k = dy * KK + dx
                pt = psum_t.tile([128, 128], FP32, name=f"pt{k}", tag="pt")
                nc.tensor.transpose(pt[:], Wt_koi[:, k, :], ident[:])
                nc.scalar.activation(
                    W_t[:, k * C:(k + 1) * C], pt[:],
                    mybir.ActivationFunctionType.Copy,
                )
                nc.vector.tensor_scalar_mul(
                    wmod[0][:, k * C:(k + 1) * C],
                    W_t[:, k * C:(k + 1) * C], scol[:, 0:1],
                )
                nc.gpsimd.tensor_mul(
                    wsq[:, k * C:(k + 1) * C],
                    W_t[:, k * C:(k + 1) * C], W_t[:, k * C:(k + 1) * C],
                )

            # ---- winv path (vector): per-o normalization, folded into the
            # drain scale; nothing here gates the conv stream
            wmax = wpool.tile([C, 1], FP32)
            nc.vector.tensor_reduce(
                wmax[:], Wt[:], axis=mybir.AxisListType.X,
                op=mybir.AluOpType.max, apply_absolute_value=True,
            )
            winv = wpool.tile([C, 1], FP32)
            nc.vector.reciprocal(winv[:], wmax[:])
            winv2 = wpool.tile([C, 1], FP32)
            nc.vector.tensor_mul(winv2[:], winv[:], winv[:])
            nc.vector.tensor_scalar_mul(winv2[:], winv2[:], 1.0 / (C * NKK))
            wfac = wpool.tile([C, 1], FP32)
            nc.vector.tensor_scalar_mul(wfac[:], winv[:], 1.0 / math.sqrt(C * NKK))

            # qt[i, o] = sum_k W_t^2 for the demod matvec
            qt = wpool.tile([C, C], FP32)
            nc.vector.tensor_reduce(
                qt[:], wsq[:].rearrange("i (k o) -> i o k", k=NKK),
                axis=mybir.AxisListType.X, op=mybir.AluOpType.add,
            )
            # sample-1 modulated weights (needed only much later)
            nc.vector.tensor_scalar_mul(wmod[1][:], W_t[:], scol[:, 1:2])
            eps_tile = wpool.tile([C, 1], FP32)
            nc.gpsimd.memset(eps_tile[:], EPS)
            coe = wpool.tile([C, S], FP32)

            # Width is handled by 1-col zero padding in the band buffer (fp32r
            # matmuls require 2D-contiguous PSUM outputs, so width clipping is
            # out); top/bottom image edges are handled by ROW clipping, which
            # keeps the output a contiguous whole-row slice.
            coe_emitted = False
            nband = 0
            for b in range(S):
                for bi in range(NB):
                    r0 = bi * BH
                    lo = max(r0 - 1, 0)
                    hi = min(r0 + BH, H - 1)
                    j0 = lo - (r0 - 1)
                    if nband == 0:
                        band = band0  # allocated + loaded up top
                        if BF16_MM:
                            bandbf = band0bf
                    else:
                        band = xpool.tile([C, BROWS, WP], FP32R, name="band", tag="band")
                        if nband < 4:
                            # pad columns are written only here; pool buffers
                            # keep them zero across rotations (the DMA touches
                            # 1:W+1 only)
                            nc.gpsimd.tensor_copy(band[:, :, 0], zeros[:])
                            nc.gpsimd.tensor_copy(band[:, :, WP - 1], zeros[:])
                        nc.sync.dma_start(
                            band[:, j0:j0 + (hi - lo + 1), 1:W + 1],
                            x_d[b, :, lo:hi + 1, :],
                        )
                        if BF16_MM:
                            # full-tile convert on the otherwise idle gpsimd:
                            # pad columns come along (zero in the fp32 tile)
                            bandbf = xbf.tile([C, BROWS, WP], BF16, name="bandbf", tag="bandbf")
                            nc.gpsimd.tensor_copy(bandbf[:], band[:])
                    if BF16_MM:
                        band = bandbf
                    nband += 1
                    last_band = (b == S - 1) and (bi == NB - 1)
                    for g in range(BH // (GT * RPT)):
                        gy = g * GT * RPT
                        ot = opool.tile([C, GT * RPT, W], FP32, name="ot", tag="ot")
                        deferred = []
                        for u in range(GT):
                            yl = gy + u * RPT
                            ps = psum_conv.tile([C, RPT * W], FP32, name="ps", tag="ps")
                            for idx, (dy, dx) in enumerate(TAPS):
                                ra = 1 if (bi == 0 and yl == 0 and dy == 0) else 0
                                rb = RPT - 1 if (bi == NB - 1 and yl == BH - RPT and dy == 2) else RPT
                                nc.tensor.matmul(
                                    ps[:, ra * W:rb * W],
                                    wmod[b][:, (dy * KK + dx) * C:(dy * KK + dx + 1) * C],
                                    band[:, yl + dy + ra:yl + dy + rb, dx:dx + W],
                                    start=(idx == 0),
                                    stop=(idx == NKK - 1),
                                    skip_group_check=True,
                                )
                            deferred.append((u, ps))
                            if not coe_emitted:
                                if u < 2:
                                    # defer early drains until coe exists
                                    continue
                                # demod scale: coe[o,b] = wfac / sqrt(winv2*Q+eps).
                                # Emitted after the second tile's matmuls (so the
                                # PE reaches the conv without waiting on qt) but
                                # before the first drains, which read coe.
                                coe_emitted = True
                                ps_coe = psum_misc.tile([C, S], FP32, tag="ps_coe")
                                nc.tensor.matmul(
                                    ps_coe[:], qt[:], scol[:, S:2 * S],
                                    start=True, stop=True,
                                )
                                nc.scalar.activation(
                                    coe[:], ps_coe[:], mybir.ActivationFunctionType.Sqrt,
                                    bias=eps_tile[:], scale=winv2[:],
                                )
                                nc.vector.reciprocal(coe[:], coe[:])
                                nc.vector.tensor_scalar_mul(coe[:], coe[:], wfac[:])
                            for ud, psd in deferred:
                                ots = ot[:, ud * RPT:(ud + 1) * RPT, :]
                                ps_r = psd[:].rearrange("c (r w) -> c r w", r=RPT)
                                if ud % 2 == 0:
                                    nc.scalar.activation(
                                        ots, ps_r, mybir.ActivationFunctionType.Copy,
                                        bias=0.0, scale=coe[:, b:b + 1],
                                    )
                                else:
                                    nc.vector.tensor_scalar_mul(
                                        ots, ps_r, coe[:, b:b + 1],
                                    )
                                if last_band:
                                    nc.sync.dma_start(
                                        out_d[b, :,
                                              r0 + gy + ud * RPT:r0 + gy + (ud + 1) * RPT, :],
                                        ots,
                                    )
                            deferred = []
                        if not last_band:
                            nc.sync.dma_start(
                                out_d[b, :, r0 + gy:r0 + gy + GT * RPT, :], ot[:],
                            )

    nc.compile()
    return nc


_CACHED = {}


def kernel(x: np.ndarray, style: np.ndarray, weight: np.ndarray, trace: bool = False):
    x = np.ascontiguousarray(x, dtype=np.float32)
    style = np.ascontiguousarray(style, dtype=np.float32)
    weight = np.ascontiguousarray(weight, dtype=np.float32)

    if "nc" not in _CACHED:
        _CACHED["nc"] = build_bass()
    nc = _CACHED["nc"]

    in_maps = [
        {
            "x": x[i * S:(i + 1) * S],
            "style": style[i * S:(i + 1) * S],
            "weight": weight,
        }
        for i in range(N_CORES)
    ]
    res = run_bass_kernel_spmd(
        nc, in_maps, core_ids=list(range(N_CORES)), trace=trace,
    )
    out = np.concatenate([r["out"] for r in res.results], axis=0)
    if trace:
        kernel.last_results = res
    return out



# revision 5
# speedup vs baseline: 1.0859x; 1.0859x over previous
"""StyleGAN2-style modulated 3x3 conv (B=16, C=128, H=W=128) on 8 TRN2 NeuronCores.

Sharding: data-parallel over batch (2 samples/core). The grouped conv runs as 9
accumulated matmuls per 4-row PSUM tile with the input-channel dim (128) as the
contraction. Spatial zero-padding is never materialized: boundary taps issue
partial-region PSUM accumulations instead, so band DMAs are fully contiguous.

Weight normalization (1/(sqrt(i*k*k) * ||w||_inf)) is folded into the per-
(sample, out-channel) demodulation scale applied at PSUM drain, so the matmul
weights are just raw-transposed weights modulated by the per-input-channel
style. Drains run on the scalar/gpsimd engines, keeping vector off the
critical path.
"""

import math
from itertools import product

import numpy as np

import concourse.bacc as bacc
import concourse.bass as bass
import concourse.mybir as mybir
import concourse.tile as tile
from concourse.bass_utils import run_bass_kernel_spmd
from concourse.masks import make_identity

B, C, H, W = 16, 128, 128, 128
KK = 3
EPS = 1e-8
N_CORES = 8
S = B // N_CORES          # samples per core
RPT = 4                   # output rows per PSUM tile (one PSUM bank)
BH = 32                   # output rows per band
NB = H // BH              # bands per sample
BROWS = BH + 2            # band rows incl. 1-row halo each side
WP = W + 2                # band width incl. 1-col zero pad each side
GT = 4                    # PSUM tiles per output store DMA
NKK = KK * KK

FP32 = mybir.dt.float32
FP32R = mybir.dt.float32r
BF16 = mybir.dt.bfloat16

BF16_MM = True            # trace-measured fp32r cadence is 282ns/512 rows (213
                          # stream + 53 high-half self-load + issue); bf16 at 259
                          # beats it. Mixed bf16-stationary/fp32r-moving is
                          # rejected by walrus (NCC_IBIR034), so both sides cast.


def build_bass() -> bass.Bass:
    w_dt = BF16 if BF16_MM else FP32R
    nc = bacc.Bacc(None)
    x_d = nc.dram_tensor("x", [S, C, H, W], FP32R, kind="ExternalInput")
    style_d = nc.dram_tensor("style", [S, C], FP32, kind="ExternalInput")
    w_d = nc.dram_tensor("weight", [C, C, KK, KK], FP32, kind="ExternalInput")
    out_d = nc.dram_tensor("out", [S, C, H, W], FP32, kind="ExternalOutput")

    with tile.TileContext(nc) as tc:
        with (
            tc.tile_pool(name="const", bufs=1) as const_pool,
            tc.tile_pool(name="wpool", bufs=1) as wpool,
            tc.tile_pool(name="xpool", bufs=4) as xpool,
            tc.tile_pool(name="xbf", bufs=4) as xbf,
            tc.tile_pool(name="opool", bufs=3) as opool,
            tc.tile_pool(name="dram", bufs=1, space="DRAM") as dram_pool,
            tc.tile_pool(name="psum_conv", bufs=4, space="PSUM") as psum_conv,
            tc.tile_pool(name="psum_t", bufs=2, space="PSUM") as psum_t,
            tc.tile_pool(name="psum_misc", bufs=1, space="PSUM") as psum_misc,
        ):
            zeros = const_pool.tile([128, BROWS], FP32)
            nc.gpsimd.memset(zeros[:], 0.0)

            # ---- input DMAs (kick off immediately) ----
            # The weight load goes through the gpsimd software DGE so it
            # transfers in parallel with the SP queue, whose order is the
            # latency order: style row, leading rows of band 0 (they gate the
            # first conv matmuls), rest of band 0.
            srow = wpool.tile([S, 2 * C], FP32)
            nc.sync.dma_start(srow[:, 0:C], style_d[:])
            Wt = wpool.tile([C, C * NKK], FP32)
            nc.gpsimd.dma_start(Wt[:], w_d[:].rearrange("o i kh kw -> o (i kh kw)"))
            # identity BEFORE the pad-column copies: both run on gpsimd, and the
            # weight transposes (which gate the whole conv stream) need ident.
            ident = const_pool.tile([128, 128], FP32)
            make_identity(nc, ident)
            band0 = xpool.tile([C, BROWS, WP], FP32R, name="band", tag="band")
            nc.gpsimd.tensor_copy(band0[:, :, 0], zeros[:])
            nc.gpsimd.tensor_copy(band0[:, :, WP - 1], zeros[:])
            nc.sync.dma_start(band0[:, 1:10, 1:W + 1], x_d[0, :, 0:9, :])
            for ja, jb in ((10, 22), (22, 34)):
                nc.sync.dma_start(band0[:, ja:jb, 1:W + 1], x_d[0, :, ja - 1:jb - 1, :])
            if BF16_MM:
                band0bf = xbf.tile([C, BROWS, WP], BF16, name="bandbf", tag="bandbf")
                nc.gpsimd.tensor_copy(band0bf[:, 0:1, :], band0[:, 0:1, :])
                for ja, jb in ((1, 10), (10, 22), (22, 34)):
                    nc.gpsimd.tensor_copy(band0bf[:, ja:jb, :], band0[:, ja:jb, :])

            # taps ordered so (1,1) — always full-size — comes first: it is the
            # start=True matmul of every accumulation group, and its weight
            # block is transposed/modulated first so the conv can start early.
            TAPS = [(1, 1)] + [t for t in product(range(KK), range(KK)) if t != (1, 1)]

            # ---- style path (vector; independent of weights) ----
            smax = wpool.tile([S, 1], FP32)
            nc.vector.tensor_reduce(
                smax[:], srow[:, 0:C], axis=mybir.AxisListType.X,
                op=mybir.AluOpType.max, apply_absolute_value=True,
            )
            sinv = wpool.tile([S, 1], FP32)
            nc.vector.reciprocal(sinv[:], smax[:])
            nc.vector.tensor_scalar_mul(srow[:, 0:C], srow[:, 0:C], sinv[:])
            nc.vector.tensor_mul(srow[:, C:2 * C], srow[:, 0:C], srow[:, 0:C])
            # on-chip transpose to column layout (the DMA queues are saturated
            # with band loads at this point, so no DRAM bounce):
            # scol[c, 0:S] = s[b, c], scol[c, S:2S] = s[b, c]^2
            scol = wpool.tile([C, 2 * S], FP32)
            for h in range(2):
                pt_s = psum_misc.tile([C, S], FP32, name=f"pts{h}", tag="pts")
                nc.tensor.transpose(
                    pt_s[:], srow[:, h * C:(h + 1) * C], ident[0:S, 0:S],
                )
                nc.scalar.activation(
                    scol[:, h * S:(h + 1) * S], pt_s[:],
                    mybir.ActivationFunctionType.Copy,
                )

            # ---- transpose raw weights tap by tap: W_t[i, k*C+o] ----
            # Each tap's transposed block is immediately modulated by the
            # sample-0 style on vector, so the conv matmuls (which consume the
            # taps in the same order) start as soon as tap (1,1) is ready.
            W_t = wpool.tile([C, NKK * C], FP32)
            wmod = [
                wpool.tile([C, NKK * C], w_dt, name=f"wmod{b}", tag=f"wmod{b}")
                for b in range(S)
            ]
            Wt_koi = Wt[:].rearrange("o (i k) -> o k i", k=NKK)
            wsq = wpool.tile([C, NKK * C], FP32)
            for dy, dx in TAPS:
                k = dy * KK + dx
                pt = psum_t.tile([128, 128], FP32, name=f"pt{k}", tag="pt")
                nc.tensor.transpose(pt[:], Wt_koi[:, k, :], ident[:])
                nc.scalar.activation(
                    W_t[:, k * C:(k + 1) * C], pt[:],
                    mybir.ActivationFunctionType.Copy,
                )
                nc.vector.tensor_scalar_mul(
                    wmod[0][:, k * C:(k + 1) * C],
                    W_t[:, k * C:(k + 1) * C], scol[:, 0:1],
                )
                nc.gpsimd.tensor_mul(
                    wsq[:, k * C:(k + 1) * C],
                    W_t[:, k * C:(k + 1) * C], W_t[:, k * C:(k + 1) * C],
                )

            # ---- winv path (vector): per-o normalization, folded into the
            # drain scale; nothing here gates the conv stream
            wmax = wpool.tile([C, 1], FP32)
            nc.vector.tensor_reduce(
                wmax[:], Wt[:], axis=mybir.AxisListType.X,
                op=mybir.AluOpType.max, apply_absolute_value=True,
            )
            winv = wpool.tile([C, 1], FP32)
            nc.vector.reciprocal(winv[:], wmax[:])
            winv2 = wpool.tile([C, 1], FP32)
            nc.vector.tensor_mul(winv2[:], winv[:], winv[:])
            nc.vector.tensor_scalar_mul(winv2[:], winv2[:], 1.0 / (C * NKK))
            wfac = wpool.tile([C, 1], FP32)
            nc.vector.tensor_scalar_mul(wfac[:], winv[:], 1.0 / math.sqrt(C * NKK))

            # qt[i, o] = sum_k W_t^2 for the demod matvec
            qt = wpool.tile([C, C], FP32)
            nc.vector.tensor_reduce(
                qt[:], wsq[:].rearrange("i (k o) -> i o k", k=NKK),
                axis=mybir.AxisListType.X, op=mybir.AluOpType.add,
            )
            # sample-1 modulated weights (needed only much later)
            nc.vector.tensor_scalar_mul(wmod[1][:], W_t[:], scol[:, 1:2])
            eps_tile = wpool.tile([C, 1], FP32)
            nc.gpsimd.memset(eps_tile[:], EPS)
            coe = wpool.tile([C, S], FP32)

            # Width is handled by 1-col zero padding in the band buffer (fp32r
            # matmuls require 2D-contiguous PSUM outputs, so width clipping is
            # out); top/bottom image edges are handled by ROW clipping, which
            # keeps the output a contiguous whole-row slice.
            coe_emitted = False
            nband = 0
            for b in range(S):
                for bi in range(NB):
                    r0 = bi * BH
                    lo = max(r0 - 1, 0)
                    hi = min(r0 + BH, H - 1)
                    j0 = lo - (r0 - 1)
                    if nband == 0:
                        band = band0  # allocated + loaded up top
                        if BF16_MM:
                            bandbf = band0bf
                    else:
                        band = xpool.tile([C, BROWS, WP], FP32R, name="band", tag="band")
                        if nband < 4:
                            # pad columns are written only here; pool buffers
                            # keep them zero across rotations (the DMA touches
                            # 1:W+1 only)
                            nc.gpsimd.tensor_copy(band[:, :, 0], zeros[:])
                            nc.gpsimd.tensor_copy(band[:, :, WP - 1], zeros[:])
                        nc.sync.dma_start(
                            band[:, j0:j0 + (hi - lo + 1), 1:W + 1],
                            x_d[b, :, lo:hi + 1, :],
                        )
                        if BF16_MM:
                            # full-tile convert on the otherwise idle gpsimd:
                            # pad columns come along (zero in the fp32 tile)
                            bandbf = xbf.tile([C, BROWS, WP], BF16, name="bandbf", tag="bandbf")
                            nc.gpsimd.tensor_copy(bandbf[:], band[:])
                    if BF16_MM:
                        band = bandbf
                    nband += 1
                    last_band = (b == S - 1) and (bi == NB - 1)
                    for g in range(BH // (GT * RPT)):
                        gy = g * GT * RPT
                        ot = opool.tile([C, GT * RPT, W], FP32, name="ot", tag="ot")
                        deferred = []
                        for u in range(GT):
                            yl = gy + u * RPT
                            ps = psum_conv.tile([C, RPT * W], FP32, name="ps", tag="ps")
                            for idx, (dy, dx) in enumerate(TAPS):
                                ra = 1 if (bi == 0 and yl == 0 and dy == 0) else 0
                                rb = RPT - 1 if (bi == NB - 1 and yl == BH - RPT and dy == 2) else RPT
                                nc.tensor.matmul(
                                    ps[:, ra * W:rb * W],
                                    wmod[b][:, (dy * KK + dx) * C:(dy * KK + dx + 1) * C],
                                    band[:, yl + dy + ra:yl + dy + rb, dx:dx + W],
                                    start=(idx == 0),
                                    stop=(idx == NKK - 1),
                                    skip_group_check=True,
                                )
                            deferred.append((u, ps))
                            if not coe_emitted:
                                if u < 2:
                                    # defer early drains until coe exists
                                    continue
                                # demod scale: coe[o,b] = wfac / sqrt(winv2*Q+eps).
                                # Emitted after the second tile's matmuls (so the
                                # PE reaches the conv without waiting on qt) but
                                # before the first drains, which read coe.
                                coe_emitted = True
                                ps_coe = psum_misc.tile([C, S], FP32, tag="ps_coe")
                                nc.tensor.matmul(
                                    ps_coe[:], qt[:], scol[:, S:2 * S],
                                    start=True, stop=True,
                                )
                                nc.scalar.activation(
                                    coe[:], ps_coe[:], mybir.ActivationFunctionType.Sqrt,
                                    bias=eps_tile[:], scale=winv2[:],
                                )
                                nc.vector.reciprocal(coe[:], coe[:])
                                nc.vector.tensor_scalar_mul(coe[:], coe[:], wfac[:])
                            for ud, psd in deferred:
                                ots = ot[:, ud * RPT:(ud + 1) * RPT, :]
                                ps_r = psd[:].rearrange("c (r w) -> c r w", r=RPT)
                                if ud % 2 == 0:
                                    nc.scalar.activation(
                                        ots, ps_r, mybir.ActivationFunctionType.Copy,
                                        bias=0.0, scale=coe[:, b:b + 1],
                                    )
                                else:
                                    nc.vector.tensor_scalar_mul(
                                        ots, ps_r, coe[:, b:b + 1],
                                    )
                                if last_band:
                                    nc.sync.dma_start(
                                        out_d[b, :,
                                              r0 + gy + ud * RPT:r0 + gy + (ud + 1) * RPT, :],
                                        ots,
                                    )
                            deferred = []
                        if not last_band:
                            nc.sync.dma_start(
                                out_d[b, :, r0 + gy:r0 + gy + GT * RPT, :], ot[:],
                            )

    nc.compile()
    return nc


_CACHED = {}


def kernel(x: np.ndarray, style: np.ndarray, weight: np.ndarray, trace: bool = False):
    x = np.ascontiguousarray(x, dtype=np.float32)
    style = np.ascontiguousarray(style, dtype=np.float32)
    weight = np.ascontiguousarray(weight, dtype=np.float32)

    if "nc" not in _CACHED:
        _CACHED["nc"] = build_bass()
    nc = _CACHED["nc"]

    in_maps = [
        {
            "x": x[i * S:(i + 1) * S],
            "style": style[i * S:(i + 1) * S],
            "weight": weight,
        }
        for i in range(N_CORES)
    ]
    res = run_bass_kernel_spmd(
        nc, in_maps, core_ids=list(range(N_CORES)), trace=trace,
    )
    out = np.concatenate([r["out"] for r in res.results], axis=0)
    if trace:
        kernel.last_results = res
    return out

